# revision 1
# baseline (speedup 1.0000x reference)
"""Chamfer distance loss kernel for Trainium2 (8 NeuronCores).

Problem: template/source [4, 8192, 3] fp32 -> scalar chamfer loss.

Sharding: 8 cores = 4 batches x 2 template-halves. Each core computes the
[4096, 8192] squared-distance matrix D between its template half and the
full source of its batch:
    d[n,m] = |t_n|^2 + |s_m|^2 - 2 t_n . s_m

The cross/source-norm terms ride a K=11 fp16 matmul (fp32 matmuls run at
~1/4 rate on trn2): u = -2t and s are split into hi/lo fp16 components
(~22 mantissa bits combined) and the three first-order cross blocks are
kept; |s|^2 is hi/lo-split into two fp16 rows against ones rows. The
template norm |t|^2 stays exact fp32 and enters via the ScalarE
activation bias (per-partition) during the PSUM->SBUF copy.

The packed operands are replicated at partition bases 0/32/64/96 and the
four column stripes use different bases, so each matmul's LDWEIGHTS
targets a different PE row-group than the in-flight matmul and overlaps
it (same-row-group LDWEIGHTS serialize).

Per D tile [128, 2048] (PSUM fp32):
  - ScalarE: out = Identity(-psum - nt[p]) cast to fp16 SBUF (negation
    turns min-reductions into max-reductions).
  - VectorE: column maxima accumulate (-> col-min of D) with fp16 2x-mode
    tensor_tensor max; row maxima via two max-folds plus one
    tensor_tensor_reduce whose accumulator gives the row max directly.
  - TensorE transposes the column accumulators (128x128 blocks) into PSUM
    so the final cross-partition reduction becomes a free-dim reduce.
  - sqrt on ScalarE (monotonic, commutes with the host-side min).

Host combine is pure gather/reduction: sum of per-core row sums plus the
elementwise min over the two half-core col-sqrt arrays, normalized.
"""

import numpy as np

B = 4
N = 8192  # template points per batch
M = 8192  # source points per batch
HALF = N // 2  # template rows per core
RB = HALF // 128  # 32 row blocks per core
STRIPES = M // 2048  # 4 col stripes of 2048
CH = 1024  # prologue chunk
K = 11  # packed contraction dim
N_CORES = 8

_CACHE = {}


def _build_bass():
    import concourse.tile as tile
    from concourse import bacc, mybir

    fp32 = mybir.dt.float32
    fp16 = mybir.dt.float16
    AF = mybir.ActivationFunctionType
    Alu = mybir.AluOpType
    X = mybir.AxisListType.X

    nc = bacc.Bacc(trn_type="TRN2")

    tmplT = nc.dram_tensor("tmplT", [3, HALF], fp32, kind="ExternalInput")
    srcT = nc.dram_tensor("srcT", [3, M], fp32, kind="ExternalInput")
    out_rowsums = nc.dram_tensor(
        "out_rowsums", [128, 1], fp32, kind="ExternalOutput"
    )
    # out_colsq[p, t] = sqrt(relu(colmin[128*t + p])), t in [0, 64)
    out_colsq = nc.dram_tensor(
        "out_colsq", [128, M // 128], fp32, kind="ExternalOutput"
    )

    # row layout of the K=11 fp16 packing (A* = components of -2t, B* = of
    # s, E* = of |s|^2):   lhsT rows      rhs rows
    #   0-2    A1                          B1
    #   3-5    A1                          B2
    #   6-8    A2                          B1
    #   9,10   ones                        E1 E2
    A_ROWS = {1: (0, 3), 2: (6,)}
    B_ROWS = {1: (0, 6), 2: (3,)}

    with tile.TileContext(nc) as tc:
        with (
            tc.tile_pool(name="singles", bufs=1) as singles,
            tc.tile_pool(name="dpool", bufs=2) as dpool,
            tc.tile_pool(name="folds", bufs=2) as folds,
            tc.tile_pool(name="psum", bufs=2, space="PSUM") as psum_pool,
            tc.tile_pool(name="dram", bufs=1, space="DRAM") as drampool,
        ):
            # persistent tiles; the operand tiles span partitions 0..96+K so
            # the packing can be replicated at bases 0/32/64/96 (row-group
            # rotation for LDWEIGHTS overlap)
            t11 = singles.tile([96 + K, HALF], fp16, tag="t11")
            s11 = singles.tile([96 + K, M], fp16, tag="s11")
            ident = singles.tile([128, 128], fp16, tag="ident")
            nc.gpsimd.memset(ident, 0.0)
            nc.gpsimd.affine_select(
                out=ident,
                in_=ident,
                compare_op=Alu.not_equal,
                fill=1.0,
                base=0,
                pattern=[[-1, 128]],
                channel_multiplier=1,
            )
            ones3 = singles.tile([3, 1], fp32, tag="ones3")
            nc.vector.memset(ones3, 1.0)
            # negnt[p, j] = -|t_{128j+p}|^2, exact fp32 (ACT bias operand)
            negnt = singles.tile([128, RB], fp32, tag="negnt")
            # acc[s][p, j] = max over row blocks of -D[128r+p, 2048s+j]
            accs = [
                singles.tile([128, 2048], fp16, tag=f"acc{s}", name=f"acc{s}")
                for s in range(STRIPES)
            ]
            negrow = singles.tile([128, RB], fp32, tag="negrow")
            red_all = singles.tile([128, M // 128], fp32, tag="red_all")

            # DRAM images of the packed operands
            t11d = drampool.tile([K, HALF], fp16, tag="t11d")
            s11d = drampool.tile([K, M], fp16, tag="s11d")

            # ---------------- prologue: build packed operands ----------------
            with tc.tile_pool(name="scr", bufs=2) as scr:
                onesrow = singles.tile([1, HALF], fp16, tag="onesrow")
                nc.vector.memset(onesrow, 1.0)
                for r in (9, 10):
                    nc.sync.dma_start(out=t11d[r : r + 1, :], in_=onesrow)

                chunks = [("t", ci) for ci in range(HALF // CH)] + [
                    ("s", ci) for ci in range(M // CH)
                ]
                for kind, ci in chunks:
                    src_ap = tmplT if kind == "t" else srcT
                    cs = slice(ci * CH, (ci + 1) * CH)
                    raw = scr.tile([3, CH], fp32, tag="raw")
                    nc.sync.dma_start(out=raw, in_=src_ap[:, cs])
                    sq = scr.tile([3, CH], fp32, tag="sq")
                    nc.scalar.activation(out=sq, in_=raw, func=AF.Square)

                    if kind == "t":
                        # template norms, exact fp32, in [128, RB] layout:
                        # one K=3 N=1 matmul per 128-row block
                        nb = CH // 128
                        ntT = psum_pool.tile([128, nb], fp32, tag="ps")
                        for jj in range(nb):
                            nc.tensor.matmul(
                                ntT[:, jj : jj + 1],
                                sq[:, jj * 128 : (jj + 1) * 128],
                                ones3[:, 0:1],
                                start=True,
                                stop=True,
                            )
                        nc.scalar.activation(
                            out=negnt[:, ci * nb : (ci + 1) * nb],
                            in_=ntT,
                            func=AF.Copy,
                            bias=0.0,
                            scale=-1.0,
                        )
                        base = scr.tile([3, CH], fp32, tag="base")
                        nc.scalar.mul(out=base, in_=raw, mul=-2.0)
                        dimg, rows = t11d, A_ROWS
                    else:
                        # source norm row, hi/lo fp16 split vs ones rows
                        nps = psum_pool.tile([1, CH], fp32, tag="ps")
                        for q in range(CH // 512):
                            nc.tensor.matmul(
                                nps[0:1, q * 512 : (q + 1) * 512],
                                ones3,
                                sq[:, q * 512 : (q + 1) * 512],
                                start=True,
                                stop=True,
                            )
                        normc = scr.tile([1, CH], fp32, tag="normc")
                        nc.scalar.copy(out=normc, in_=nps)
                        e1 = scr.tile([1, CH], fp16, tag="e1")
                        nc.scalar.copy(out=e1, in_=normc)
                        nc.sync.dma_start(out=s11d[9:10, cs], in_=e1)
                        e2 = scr.tile([1, CH], fp16, tag="e2")
                        nc.vector.tensor_sub(e2, normc, e1)
                        nc.sync.dma_start(out=s11d[10:11, cs], in_=e2)
                        base = raw
                        dimg, rows = s11d, B_ROWS

                    # hi/lo fp16 split of the coordinate block
                    c1 = scr.tile([3, CH], fp16, tag="c1")
                    nc.scalar.copy(out=c1, in_=base)
                    for r in rows[1]:
                        nc.sync.dma_start(out=dimg[r : r + 3, cs], in_=c1)
                    c2 = scr.tile([3, CH], fp16, tag="c2")
                    nc.vector.tensor_sub(c2, base, c1)
                    for r in rows[2]:
                        nc.sync.dma_start(out=dimg[r : r + 3, cs], in_=c2)

                # load the packed operands, replicated at 4 partition bases
                for g in range(4):
                    nc.sync.dma_start(out=t11[32 * g : 32 * g + K, :], in_=t11d)
                    nc.sync.dma_start(out=s11[32 * g : 32 * g + K, :], in_=s11d)


            # ---------------- main loop ----------------
            for j in range(RB):
                d_tiles = []
                for s in range(STRIPES):
                    ps = psum_pool.tile([128, 2048], fp32, tag="ps")
                    for q in range(4):
                        # rotate the PE row group every matmul so each
                        # LDWEIGHTS overlaps the in-flight matmul
                        g = 32 * q
                        nc.tensor.matmul(
                            ps[:, q * 512 : (q + 1) * 512],
                            t11[g : g + K, j * 128 : (j + 1) * 128],
                            s11[
                                g : g + K,
                                s * 2048 + q * 512 : s * 2048 + (q + 1) * 512,
                            ],
                            start=True,
                            stop=True,
                            tile_position=(g, 0),
                        )
                    d16 = dpool.tile([128, 2048], fp16, tag=f"d{s}")
                    # d16 = -(psum + nt[p]) = -d, cast to fp16
                    nc.scalar.activation(
                        out=d16,
                        in_=ps,
                        func=AF.Identity,
                        bias=negnt[:, j : j + 1],
                        scale=-1.0,
                    )
                    d_tiles.append(d16)
                    # col accumulate (max of negated = -min)
                    if j == 0:
                        nc.vector.tensor_copy(accs[s], d16)
                    else:
                        nc.vector.tensor_tensor(accs[s], accs[s], d16, op=Alu.max)

                # row max: two pair-folds, then a fused max-fold whose
                # accumulator output is the full row max
                f01 = folds.tile([128, 2048], fp16, tag="f01")
                f23 = folds.tile([128, 2048], fp16, tag="f23")
                nc.vector.tensor_tensor(f01, d_tiles[0], d_tiles[1], op=Alu.max)
                nc.vector.tensor_tensor(f23, d_tiles[2], d_tiles[3], op=Alu.max)
                nc.vector.tensor_tensor(f01, f01, f23, op=Alu.max)
                fh = folds.tile([128, 1024], fp16, tag="fh")
                nc.vector.tensor_tensor(
                    fh, f01[:, 0:1024], f01[:, 1024:2048], op=Alu.max
                )
                fq = folds.tile([128, 512], fp16, tag="fq")
                nc.vector.tensor_tensor(
                    fq, fh[:, 0:512], fh[:, 512:1024], op=Alu.max
                )
                nc.vector.tensor_reduce(
                    negrow[:, j : j + 1], fq, axis=X, op=Alu.max
                )

            # ---------------- epilogue ----------------
            # rowmin side: clamp, sqrt, accumulate-sum along free dim
            rowclamp = singles.tile([128, RB], fp32, tag="rowclamp")
            nc.vector.tensor_scalar(
                out=rowclamp,
                in0=negrow,
                scalar1=-1.0,
                scalar2=0.0,
                op0=Alu.mult,
                op1=Alu.max,
            )
            rowsqrt = singles.tile([128, RB], fp32, tag="rowsqrt")
            rowsum = singles.tile([128, 1], fp32, tag="rowsum")
            nc.scalar.activation(
                out=rowsqrt, in_=rowclamp, func=AF.Sqrt, accum_out=rowsum
            )
            nc.sync.dma_start(out=out_rowsums[:, :], in_=rowsum)

            # colmin side: TensorE-transpose each acc stripe into PSUM, then
            # free-dim reduce does the cross-partition max.
            for s in range(STRIPES):
                psT = psum_pool.tile([128, 16, 128], fp16, tag="ps")
                for t in range(16):
                    nc.tensor.transpose(
                        psT[:, t, :], accs[s][:, t * 128 : (t + 1) * 128], ident
                    )
                nc.vector.tensor_reduce(
                    red_all[:, s * 16 : (s + 1) * 16], psT, axis=X, op=Alu.max
                )

            colclamp = singles.tile([128, M // 128], fp32, tag="colclamp")
            nc.vector.tensor_scalar(
                out=colclamp,
                in0=red_all,
                scalar1=-1.0,
                scalar2=0.0,
                op0=Alu.mult,
                op1=Alu.max,
            )
            colsqrt = singles.tile([128, M // 128], fp32, tag="colsqrt")
            nc.scalar.activation(out=colsqrt, in_=colclamp, func=AF.Sqrt)
            nc.sync.dma_start(out=out_colsq[:, :], in_=colsqrt)

    nc.compile()
    return nc


def _get_nc():
    if "nc" not in _CACHE:
        _CACHE["nc"] = _build_bass()
    return _CACHE["nc"]


def _make_in_maps(template, source):
    template = np.asarray(template, dtype=np.float32)
    source = np.asarray(source, dtype=np.float32)
    in_maps = []
    for c in range(N_CORES):
        b, h = divmod(c, 2)
        tmpl_half = template[b, h * HALF : (h + 1) * HALF, :]  # [HALF, 3]
        in_maps.append(
            {
                "tmplT": np.ascontiguousarray(tmpl_half.T),  # [3, HALF]
                "srcT": np.ascontiguousarray(source[b].T),  # [3, M]
            }
        )
    return in_maps


def _combine(results):
    # results: list of 8 dicts with out_rowsums [128,1], out_colsq [128, M//128]
    row_total = 0.0
    col_total = 0.0
    for b in range(B):
        r0 = results[2 * b]
        r1 = results[2 * b + 1]
        row_total += float(np.sum(r0["out_rowsums"], dtype=np.float64))
        row_total += float(np.sum(r1["out_rowsums"], dtype=np.float64))
        # colsq[p, t] = sqrt(relu(colmin[128 t + p])); combine halves by min
        c = np.minimum(r0["out_colsq"], r1["out_colsq"])
        col_total += float(np.sum(c, dtype=np.float64))
    loss = (row_total + col_total) / (2.0 * B * float(N))
    return np.float32(loss)


def _run_on_cores(in_maps, trace=False, **kwargs):
    from concourse.bass_utils import run_bass_kernel_spmd

    nc = _get_nc()
    return run_bass_kernel_spmd(
        nc, in_maps, core_ids=list(range(N_CORES)), trace=trace, **kwargs
    )


def kernel(template, source):
    in_maps = _make_in_maps(template, source)
    res = _run_on_cores(in_maps, trace=False)
    return _combine(res.results)



# revision 5
# speedup vs baseline: 1.2362x; 1.2362x over previous
"""Chamfer distance loss kernel for Trainium2 (8 NeuronCores).

Problem: template/source [4, 8192, 3] fp32 -> scalar chamfer loss.

Sharding: 8 cores = 4 batches x 2 template-halves. Each core computes the
[4096, 8192] squared-distance matrix D between its template half and the
full source of its batch:
    d[n,m] = |t_n|^2 + |s_m|^2 - 2 t_n . s_m

All K=13 terms ride a single fp16 matmul so PSUM holds the COMPLETE D:
the three first-order cross blocks of the hi/lo fp16 split of u=-2t and
s (~22 mantissa bits combined), |s|^2 hi/lo against template-side ones
rows, and |t|^2 hi/lo against source-side ones rows. The packed operands
(including the norms and hi/lo splits, which are O(N) work) are built on
the HOST in numpy - the device prologue is just 8 replica DMA loads.

The packed operands are replicated at partition bases 0/32/64/96 and the
four 512-column sub-matmuls of each stripe use different bases, so the
matmuls run concurrently in distinct PE row groups.

Per D tile [128, 2048] (PSUM fp32), j = template row block, s = stripe:
  - ScalarE: d16 = fp16(psum) - a pure cast (no bias needed).
  - VectorE: column minima accumulate per stripe with fp16 2x-mode
    tensor_tensor min; row minima via two fused tensor_tensor_reduce ops
    (out = min(d_a, d_b) to scratch, accum_out = free-dim min) writing
    rowA[:, j] / rowB[:, j].
  - TensorE transposes the column accumulators (128x128 blocks) into
    PSUM so the final cross-partition reduction becomes a free-dim
    reduce; sqrt on ScalarE (monotonic, commutes with host-side min).

Host combine is pure gather/reduction: sum of per-core row sums plus the
elementwise min over the two half-core col-sqrt arrays, normalized.
"""

import numpy as np

B = 4
N = 8192  # template points per batch
M = 8192  # source points per batch
HALF = N // 2  # template rows per core
RB = HALF // 128  # 32 row blocks per core
STRIPES = M // 2048  # 4 col stripes of 2048
K = 13  # packed contraction dim
N_CORES = 8
BIG = 60000.0  # > any real distance, < fp16 max

_CACHE = {}


def _build_bass():
    import concourse.tile as tile
    from concourse import bacc, mybir

    fp32 = mybir.dt.float32
    fp16 = mybir.dt.float16
    AF = mybir.ActivationFunctionType
    Alu = mybir.AluOpType
    X = mybir.AxisListType.X

    nc = bacc.Bacc(trn_type="TRN2")

    t13d = nc.dram_tensor("t13", [K, HALF], fp16, kind="ExternalInput")
    s13d = nc.dram_tensor("s13", [K, M], fp16, kind="ExternalInput")
    out_rowsums = nc.dram_tensor(
        "out_rowsums", [128, 1], fp32, kind="ExternalOutput"
    )
    # out_colsq[p, t] = sqrt(relu(colmin[128*t + p])), t in [0, 64)
    out_colsq = nc.dram_tensor(
        "out_colsq", [128, M // 128], fp32, kind="ExternalOutput"
    )

    with tile.TileContext(nc) as tc:
        with (
            tc.tile_pool(name="singles", bufs=1) as singles,
            tc.tile_pool(name="dpool", bufs=2) as dpool,
            tc.tile_pool(name="folds", bufs=2) as folds,
            tc.tile_pool(name="psum", bufs=2, space="PSUM") as psum_pool,
        ):
            # packed operands, replicated at partition bases 0/32/64/96 so
            # the four sub-matmuls of a stripe target distinct PE row groups
            t13 = singles.tile([96 + K, HALF], fp16, tag="t13")
            s13 = singles.tile([96 + K, M], fp16, tag="s13")
            for g in range(4):
                nc.sync.dma_start(out=t13[32 * g : 32 * g + K, :], in_=t13d[:, :])
                nc.sync.dma_start(out=s13[32 * g : 32 * g + K, :], in_=s13d[:, :])

            ident = singles.tile([128, 128], fp16, tag="ident")
            nc.gpsimd.memset(ident, 0.0)
            nc.gpsimd.affine_select(
                out=ident,
                in_=ident,
                compare_op=Alu.not_equal,
                fill=1.0,
                base=0,
                pattern=[[-1, 128]],
                channel_multiplier=1,
            )

            # acc[p, 2048s+i] = min over row blocks of D[128r+p, 2048s+i]
            acc = singles.tile([128, M], fp16, tag="acc")
            rowmin = singles.tile([128, RB], fp32, tag="rowmin")
            red_all = singles.tile([128, M // 128], fp32, tag="red_all")

            # ---------------- main loop ----------------
            for j in range(RB):
                d_all = dpool.tile([128, M], fp16, tag="d_all")
                for s in range(STRIPES):
                    ps = psum_pool.tile([128, 2048], fp32, tag="ps")
                    for q in range(4):
                        g = 32 * q
                        nc.tensor.matmul(
                            ps[:, q * 512 : (q + 1) * 512],
                            t13[g : g + K, j * 128 : (j + 1) * 128],
                            s13[
                                g : g + K,
                                s * 2048 + q * 512 : s * 2048 + (q + 1) * 512,
                            ],
                            start=True,
                            stop=True,
                            tile_position=(g, 0),
                        )
                    nc.scalar.copy(
                        out=d_all[:, s * 2048 : (s + 1) * 2048], in_=ps
                    )

                # column minima accumulate: one wide fp16 2x tensor_tensor
                if j == 0:
                    nc.vector.tensor_copy(acc, d_all)
                else:
                    nc.vector.tensor_tensor(acc, acc, d_all, op=Alu.min)

                # row minima: pairwise fold tree, wide fp16 2x ops
                g1 = folds.tile([128, M // 2], fp16, tag="g1")
                nc.vector.tensor_tensor(
                    g1, d_all[:, : M // 2], d_all[:, M // 2 :], op=Alu.min
                )
                g2 = folds.tile([128, M // 4], fp16, tag="g2")
                nc.vector.tensor_tensor(
                    g2, g1[:, : M // 4], g1[:, M // 4 :], op=Alu.min
                )
                g3 = folds.tile([128, M // 8], fp16, tag="g3")
                nc.vector.tensor_tensor(
                    g3, g2[:, : M // 8], g2[:, M // 8 :], op=Alu.min
                )
                g4 = folds.tile([128, M // 16], fp16, tag="g4")
                nc.vector.tensor_tensor(
                    g4, g3[:, : M // 16], g3[:, M // 16 :], op=Alu.min
                )
                nc.vector.tensor_reduce(
                    rowmin[:, j : j + 1], g4, axis=X, op=Alu.min
                )

            # ---------------- epilogue ----------------
            # row side: clamp, sqrt, accumulate-sum
            rowclamp = singles.tile([128, RB], fp32, tag="rowclamp")
            nc.vector.tensor_scalar(
                out=rowclamp,
                in0=rowmin,
                scalar1=0.0,
                scalar2=None,
                op0=Alu.max,
            )
            rowsqrt = singles.tile([128, RB], fp32, tag="rowsqrt")
            rowsum = singles.tile([128, 1], fp32, tag="rowsum")
            nc.scalar.activation(
                out=rowsqrt, in_=rowclamp, func=AF.Sqrt, accum_out=rowsum
            )
            nc.sync.dma_start(out=out_rowsums[:, :], in_=rowsum)

            # col side: TensorE-transpose acc in 128-column blocks into PSUM,
            # then free-dim reduce does the cross-partition min.
            for s in range(STRIPES):
                psT = psum_pool.tile([128, 16, 128], fp16, tag="ps")
                for t in range(16):
                    blk = s * 16 + t
                    nc.tensor.transpose(
                        psT[:, t, :], acc[:, blk * 128 : (blk + 1) * 128], ident
                    )
                nc.vector.tensor_reduce(
                    red_all[:, s * 16 : (s + 1) * 16], psT, axis=X, op=Alu.min
                )

            colclamp = singles.tile([128, M // 128], fp32, tag="colclamp")
            nc.vector.tensor_scalar(
                out=colclamp,
                in0=red_all,
                scalar1=0.0,
                scalar2=None,
                op0=Alu.max,
            )
            colsqrt = singles.tile([128, M // 128], fp32, tag="colsqrt")
            nc.scalar.activation(out=colsqrt, in_=colclamp, func=AF.Sqrt)
            nc.sync.dma_start(out=out_colsq[:, :], in_=colsqrt)

    nc.compile()
    return nc


def _get_nc():
    if "nc" not in _CACHE:
        _CACHE["nc"] = _build_bass()
    return _CACHE["nc"]


def _pack_operands(t, s):
    """Host-side O(N) packing: hi/lo fp16 splits + norms + ones rows.

    t: [HALF, 3] template slice, s: [M, 3] source (both fp32).
    Returns t13 [13, HALF], s13 [13, M] fp16 with row pairing:
        t13        s13        product
      0-2  A1      B1         hi(-2t) . hi(s)
      3-5  A1      B2         hi(-2t) . lo(s)
      6-8  A2      B1         lo(-2t) . hi(s)
      9-10 ones    E1,E2      |s|^2 hi+lo
      11-12 nth,ntl ones      |t|^2 hi+lo
    """
    u = (-2.0 * t).T.astype(np.float32)  # [3, HALF]
    A1 = u.astype(np.float16)
    A2 = (u - A1.astype(np.float32)).astype(np.float16)
    nt = np.sum(t * t, axis=1, dtype=np.float32)  # [HALF]
    nth = nt.astype(np.float16)
    ntl = (nt - nth.astype(np.float32)).astype(np.float16)

    sv = s.T.astype(np.float32)  # [3, M]
    B1 = sv.astype(np.float16)
    B2 = (sv - B1.astype(np.float32)).astype(np.float16)
    ns = np.sum(s * s, axis=1, dtype=np.float32)  # [M]
    E1 = ns.astype(np.float16)
    E2 = (ns - E1.astype(np.float32)).astype(np.float16)

    ones_t = np.ones((2, t.shape[0]), dtype=np.float16)
    ones_s = np.ones((2, s.shape[0]), dtype=np.float16)
    t13 = np.concatenate(
        [A1, A1, A2, ones_t, nth[None, :], ntl[None, :]], axis=0
    )
    s13 = np.concatenate([B1, B2, B1, E1[None, :], E2[None, :], ones_s], axis=0)
    return np.ascontiguousarray(t13), np.ascontiguousarray(s13)


def _make_in_maps(template, source):
    template = np.asarray(template, dtype=np.float32)
    source = np.asarray(source, dtype=np.float32)
    in_maps = []
    for c in range(N_CORES):
        b, h = divmod(c, 2)
        tmpl_half = template[b, h * HALF : (h + 1) * HALF, :]  # [HALF, 3]
        t13, s13 = _pack_operands(tmpl_half, source[b])
        in_maps.append({"t13": t13, "s13": s13})
    return in_maps


def _combine(results):
    # results: list of 8 dicts with out_rowsums [128,1], out_colsq [128, M//128]
    row_total = 0.0
    col_total = 0.0
    for b in range(B):
        r0 = results[2 * b]
        r1 = results[2 * b + 1]
        row_total += float(np.sum(r0["out_rowsums"], dtype=np.float64))
        row_total += float(np.sum(r1["out_rowsums"], dtype=np.float64))
        # colsq[p, t] = sqrt(relu(colmin[128 t + p])); combine halves by min
        c = np.minimum(r0["out_colsq"], r1["out_colsq"])
        col_total += float(np.sum(c, dtype=np.float64))
    loss = (row_total + col_total) / (2.0 * B * float(N))
    return np.float32(loss)


def _run_on_cores(in_maps, trace=False, **kwargs):
    from concourse.bass_utils import run_bass_kernel_spmd

    nc = _get_nc()
    return run_bass_kernel_spmd(
        nc, in_maps, core_ids=list(range(N_CORES)), trace=trace, **kwargs
    )


def kernel(template, source):
    in_maps = _make_in_maps(template, source)
    res = _run_on_cores(in_maps, trace=False)
    return _combine(res.results)


# revision 8
# speedup vs baseline: 1.3297x; 1.0757x over previous
"""Chamfer distance loss kernel for Trainium2 (8 NeuronCores).

Problem: template/source [4, 8192, 3] fp32 -> scalar chamfer loss.

Sharding: 8 cores = 4 batches x 2 template-halves. Each core computes the
[4096, 8192] squared-distance matrix D between its template half and the
full source of its batch:
    d[n,m] = |t_n|^2 + |s_m|^2 - 2 t_n . s_m

All K=13 terms ride a single fp16 matmul so PSUM holds the COMPLETE D:
the three first-order cross blocks of the hi/lo fp16 split of u=-2t and
s (~22 mantissa bits combined), |s|^2 hi/lo against template-side ones
rows, and |t|^2 hi/lo against source-side ones rows. The packed operands
(including the norms and hi/lo splits, which are O(N) work) are built on
the HOST in numpy - the device prologue is just 8 replica DMA loads.

The packed operands are replicated at partition bases 0/32/64/96 and the
four 512-column sub-matmuls of each stripe use different bases, so the
matmuls run concurrently in distinct PE row groups.

Per D tile [128, 2048] (PSUM fp32), j = template row block, s = stripe:
  - ScalarE: d16 = fp16(psum) - a pure cast (no bias needed).
  - VectorE: column minima accumulate per stripe with fp16 2x-mode
    tensor_tensor min; row minima via two fused tensor_tensor_reduce ops
    (out = min(d_a, d_b) to scratch, accum_out = free-dim min) writing
    rowA[:, j] / rowB[:, j].
  - TensorE transposes the column accumulators (128x128 blocks) into
    PSUM so the final cross-partition reduction becomes a free-dim
    reduce; sqrt on ScalarE (monotonic, commutes with host-side min).

Host combine is pure gather/reduction: sum of per-core row sums plus the
elementwise min over the two half-core col-sqrt arrays, normalized.
"""

import numpy as np

B = 4
N = 8192  # template points per batch
M = 8192  # source points per batch
HALF = N // 2  # template rows per core
RB = HALF // 128  # 32 row blocks per core
STRIPES = M // 2048  # 4 col stripes of 2048
K = 13  # packed contraction dim
N_CORES = 8
BIG = 60000.0  # > any real distance, < fp16 max

_CACHE = {}


def _register_min2r():
    """Register a fused custom DVE op: out = min(in0, in1) elementwise,
    accum_out = min(s0, min over free dim of out). One instruction reduces
    two [128, 4096] fp16 tiles to a per-partition row minimum (~4.4us),
    replacing a five-op fold tree (~5.5us)."""
    import concourse.dve_ops as dve_ops
    from concourse.dve_spec import Spec, Src0, Src1, minn, C0, lower, AluOp
    from concourse.dve_uop import DveOpSpec

    name = "MIN2R_CHAMFER"
    for o in dve_ops.OPS:
        if o.name == name:
            return o
    row = max(dve_ops._SUB_OPCODE_FOR_NAME.values()) + 1
    assert row < 0x20
    spec = Spec(body=minn(Src0, Src1), accum=AluOp.MIN, accum_init=C0)
    dve_ops._SUB_OPCODE_FOR_NAME[name] = row
    shas = {}
    for ver in ("v3", "v4"):
        tmp = DveOpSpec(
            name=name, opcode=row, uops=lower(spec, ver=ver), rd1_en=True
        )
        shas[ver] = tmp.sha(ver)
    op = dve_ops.DveOp(name, spec, subdim=False, uops_sha=shas)
    dve_ops.OPS.append(op)
    dve_ops.CUSTOM_DVE_SPECS[name] = spec
    return op


def _build_bass():
    import concourse.tile as tile
    from concourse import bacc, mybir

    fp32 = mybir.dt.float32
    fp16 = mybir.dt.float16
    AF = mybir.ActivationFunctionType
    Alu = mybir.AluOpType
    X = mybir.AxisListType.X

    min2r = _register_min2r()
    nc = bacc.Bacc(trn_type="TRN2")

    t13d = nc.dram_tensor("t13", [K, HALF], fp16, kind="ExternalInput")
    s13d = nc.dram_tensor("s13", [K, M], fp16, kind="ExternalInput")
    out_rowsums = nc.dram_tensor(
        "out_rowsums", [128, 1], fp32, kind="ExternalOutput"
    )
    # out_colsq[p, t] = sqrt(relu(colmin[128*t + p])), t in [0, 64)
    out_colsq = nc.dram_tensor(
        "out_colsq", [128, M // 128], fp32, kind="ExternalOutput"
    )

    with tile.TileContext(nc) as tc:
        with (
            tc.tile_pool(name="singles", bufs=1) as singles,
            tc.tile_pool(name="dpool", bufs=2) as dpool,
            tc.tile_pool(name="folds", bufs=2) as folds,
            tc.tile_pool(name="psum", bufs=2, space="PSUM") as psum_pool,
        ):
            # packed operands, replicated at partition bases 0/32/64/96 so
            # the four sub-matmuls of a stripe target distinct PE row groups
            t13 = singles.tile([96 + K, HALF], fp16, tag="t13")
            s13 = singles.tile([96 + K, M], fp16, tag="s13")
            # split the replica loads across both HWDGE queues (SP + ACT)
            for g in range(4):
                eng = nc.sync if g % 2 == 0 else nc.scalar
                eng.dma_start(out=t13[32 * g : 32 * g + K, :], in_=t13d[:, :])
                eng2 = nc.scalar if g % 2 == 0 else nc.sync
                eng2.dma_start(out=s13[32 * g : 32 * g + K, :], in_=s13d[:, :])

            ident = singles.tile([128, 128], fp16, tag="ident")
            nc.gpsimd.memset(ident, 0.0)
            nc.gpsimd.affine_select(
                out=ident,
                in_=ident,
                compare_op=Alu.not_equal,
                fill=1.0,
                base=0,
                pattern=[[-1, 128]],
                channel_multiplier=1,
            )

            # acc[p, 2048s+i] = min over row blocks of D[128r+p, 2048s+i]
            acc = singles.tile([128, M], fp16, tag="acc")
            rowmin = singles.tile([128, RB], fp32, tag="rowmin")
            red_all = singles.tile([128, M // 128], fp32, tag="red_all")

            # ---------------- main loop ----------------
            for j in range(RB):
                d_all = dpool.tile([128, M], fp16, tag="d_all")
                for s in range(STRIPES):
                    ps = psum_pool.tile([128, 2048], fp32, tag="ps")
                    for q in range(4):
                        g = 32 * q
                        nc.tensor.matmul(
                            ps[:, q * 512 : (q + 1) * 512],
                            t13[g : g + K, j * 128 : (j + 1) * 128],
                            s13[
                                g : g + K,
                                s * 2048 + q * 512 : s * 2048 + (q + 1) * 512,
                            ],
                            start=True,
                            stop=True,
                            tile_position=(g, 0),
                        )
                    nc.scalar.copy(
                        out=d_all[:, s * 2048 : (s + 1) * 2048], in_=ps
                    )

                # column minima accumulate: one wide fp16 2x tensor_tensor.
                # On the last iteration accumulate per stripe instead, so
                # each stripe's epilogue transposes can start while the
                # remaining stripes are still accumulating.
                if j == 0:
                    nc.vector.tensor_copy(acc, d_all)
                elif j < RB - 1:
                    nc.vector.tensor_tensor(acc, acc, d_all, op=Alu.min)
                else:
                    for s in range(STRIPES):
                        cs = slice(s * 2048, (s + 1) * 2048)
                        nc.vector.tensor_tensor(
                            acc[:, cs], acc[:, cs], d_all[:, cs], op=Alu.min
                        )

                # row minima: one fused custom DVE op (min of the two tile
                # halves elementwise, with a min-reduce accumulator)
                g1 = folds.tile([128, M // 2], fp16, tag="g1")
                nc.vector._custom_dve(
                    min2r,
                    out=g1,
                    accum_out=rowmin[:, j : j + 1],
                    in0=d_all[:, : M // 2],
                    in1=d_all[:, M // 2 :],
                    s0=BIG,
                )

            # ---------------- epilogue ----------------
            # row side: clamp, sqrt, accumulate-sum
            rowclamp = singles.tile([128, RB], fp32, tag="rowclamp")
            nc.vector.tensor_scalar(
                out=rowclamp,
                in0=rowmin,
                scalar1=0.0,
                scalar2=None,
                op0=Alu.max,
            )
            rowsqrt = singles.tile([128, RB], fp32, tag="rowsqrt")
            rowsum = singles.tile([128, 1], fp32, tag="rowsum")
            nc.scalar.activation(
                out=rowsqrt, in_=rowclamp, func=AF.Sqrt, accum_out=rowsum
            )
            nc.sync.dma_start(out=out_rowsums[:, :], in_=rowsum)

            # col side: TensorE-transpose acc in 128-column blocks into PSUM,
            # then free-dim reduce does the cross-partition min.
            for s in range(STRIPES):
                psT = psum_pool.tile([128, 16, 128], fp16, tag="ps")
                for t in range(16):
                    blk = s * 16 + t
                    nc.tensor.transpose(
                        psT[:, t, :], acc[:, blk * 128 : (blk + 1) * 128], ident
                    )
                nc.vector.tensor_reduce(
                    red_all[:, s * 16 : (s + 1) * 16], psT, axis=X, op=Alu.min
                )

            colclamp = singles.tile([128, M // 128], fp32, tag="colclamp")
            nc.vector.tensor_scalar(
                out=colclamp,
                in0=red_all,
                scalar1=0.0,
                scalar2=None,
                op0=Alu.max,
            )
            colsqrt = singles.tile([128, M // 128], fp32, tag="colsqrt")
            nc.scalar.activation(out=colsqrt, in_=colclamp, func=AF.Sqrt)
            nc.sync.dma_start(out=out_colsq[:, :], in_=colsqrt)

    nc.compile()
    return nc


def _get_nc():
    if "nc" not in _CACHE:
        _CACHE["nc"] = _build_bass()
    return _CACHE["nc"]


def _pack_operands(t, s):
    """Host-side O(N) packing: hi/lo fp16 splits + norms + ones rows.

    t: [HALF, 3] template slice, s: [M, 3] source (both fp32).
    Returns t13 [13, HALF], s13 [13, M] fp16 with row pairing:
        t13        s13        product
      0-2  A1      B1         hi(-2t) . hi(s)
      3-5  A1      B2         hi(-2t) . lo(s)
      6-8  A2      B1         lo(-2t) . hi(s)
      9-10 ones    E1,E2      |s|^2 hi+lo
      11-12 nth,ntl ones      |t|^2 hi+lo
    """
    u = (-2.0 * t).T.astype(np.float32)  # [3, HALF]
    A1 = u.astype(np.float16)
    A2 = (u - A1.astype(np.float32)).astype(np.float16)
    nt = np.sum(t * t, axis=1, dtype=np.float32)  # [HALF]
    nth = nt.astype(np.float16)
    ntl = (nt - nth.astype(np.float32)).astype(np.float16)

    sv = s.T.astype(np.float32)  # [3, M]
    B1 = sv.astype(np.float16)
    B2 = (sv - B1.astype(np.float32)).astype(np.float16)
    ns = np.sum(s * s, axis=1, dtype=np.float32)  # [M]
    E1 = ns.astype(np.float16)
    E2 = (ns - E1.astype(np.float32)).astype(np.float16)

    ones_t = np.ones((2, t.shape[0]), dtype=np.float16)
    ones_s = np.ones((2, s.shape[0]), dtype=np.float16)
    t13 = np.concatenate(
        [A1, A1, A2, ones_t, nth[None, :], ntl[None, :]], axis=0
    )
    s13 = np.concatenate([B1, B2, B1, E1[None, :], E2[None, :], ones_s], axis=0)
    return np.ascontiguousarray(t13), np.ascontiguousarray(s13)


def _make_in_maps(template, source):
    template = np.asarray(template, dtype=np.float32)
    source = np.asarray(source, dtype=np.float32)
    in_maps = []
    for c in range(N_CORES):
        b, h = divmod(c, 2)
        tmpl_half = template[b, h * HALF : (h + 1) * HALF, :]  # [HALF, 3]
        t13, s13 = _pack_operands(tmpl_half, source[b])
        in_maps.append({"t13": t13, "s13": s13})
    return in_maps


def _combine(results):
    # results: list of 8 dicts with out_rowsums [128,1], out_colsq [128, M//128]
    row_total = 0.0
    col_total = 0.0
    for b in range(B):
        r0 = results[2 * b]
        r1 = results[2 * b + 1]
        row_total += float(np.sum(r0["out_rowsums"], dtype=np.float64))
        row_total += float(np.sum(r1["out_rowsums"], dtype=np.float64))
        # colsq[p, t] = sqrt(relu(colmin[128 t + p])); combine halves by min
        c = np.minimum(r0["out_colsq"], r1["out_colsq"])
        col_total += float(np.sum(c, dtype=np.float64))
    loss = (row_total + col_total) / (2.0 * B * float(N))
    return np.float32(loss)


def _run_on_cores(in_maps, trace=False, **kwargs):
    from concourse.bass_utils import run_bass_kernel_spmd

    nc = _get_nc()
    return run_bass_kernel_spmd(
        nc, in_maps, core_ids=list(range(N_CORES)), trace=trace, **kwargs
    )


def kernel(template, source):
    in_maps = _make_in_maps(template, source)
    res = _run_on_cores(in_maps, trace=False)
    return _combine(res.results)


# revision 9
# speedup vs baseline: 1.3386x; 1.0067x over previous
"""Chamfer distance loss kernel for Trainium2 (8 NeuronCores).

Problem: template/source [4, 8192, 3] fp32 -> scalar chamfer loss.

Sharding: 8 cores = 4 batches x 2 template-halves. Each core computes the
[4096, 8192] squared-distance matrix D between its template half and the
full source of its batch:
    d[n,m] = |t_n|^2 + |s_m|^2 - 2 t_n . s_m

All K=13 terms ride a single fp16 matmul so PSUM holds the COMPLETE D:
the three first-order cross blocks of the hi/lo fp16 split of u=-2t and
s (~22 mantissa bits combined), |s|^2 hi/lo against template-side ones
rows, and |t|^2 hi/lo against source-side ones rows. The packed operand
image (norms + hi/lo splits are O(N) work) is built on the HOST in
numpy; the device prologue is 4 replica DMA loads.

The packed image is replicated at partition bases 0/32/64/96 and the
four 512-column sub-matmuls of each stripe use different bases, so the
matmuls run concurrently in distinct PE row groups.

Main loop per template row block j (32 iterations, [128, 8192] D row):
  - ScalarE: d16 = fp16(psum), four [128, 2048] casts (pure drain).
  - VectorE: one wide fp16 2x tensor_tensor min accumulates column
    minima; one custom fused DVE op (MIN2R: out = min(lo, hi) halves,
    accum_out = free-dim min) produces the complete row minimum.
  - TensorE epilogue: transpose the column accumulator as BITCAST fp32
    (halves the transpose count; fp16 pairs ride as fp32 bit patterns)
    into PSUM; a strided free-dim reduce does the cross-partition min.

Host combine: sqrt/clamp/sums of the tiny per-core min arrays (O(N)),
plus the elementwise min over the two half-core column arrays.
"""

import numpy as np

B = 4
N = 8192  # template points per batch
M = 8192  # source points per batch
HALF = N // 2  # template rows per core
RB = HALF // 128  # 32 row blocks per core
STRIPES = M // 2048  # 4 col stripes of 2048
K = 13  # packed contraction dim
TS = HALF + M  # fused operand image columns (template then source)
N_CORES = 8
BIG = 60000.0  # > any real distance, < fp16 max

_CACHE = {}


def _register_min2r():
    """Register a fused custom DVE op: out = min(in0, in1) elementwise,
    accum_out = min(s0, min over free dim of out). One instruction reduces
    two [128, 4096] fp16 tiles to a per-partition row minimum (~4.4us),
    replacing a five-op fold tree (~5.5us)."""
    import concourse.dve_ops as dve_ops
    from concourse.dve_spec import Spec, Src0, Src1, minn, C0, lower, AluOp
    from concourse.dve_uop import DveOpSpec

    name = "MIN2R_CHAMFER"
    for o in dve_ops.OPS:
        if o.name == name:
            return o
    row = max(dve_ops._SUB_OPCODE_FOR_NAME.values()) + 1
    assert row < 0x20
    spec = Spec(body=minn(Src0, Src1), accum=AluOp.MIN, accum_init=C0)
    dve_ops._SUB_OPCODE_FOR_NAME[name] = row
    shas = {}
    for ver in ("v3", "v4"):
        tmp = DveOpSpec(
            name=name, opcode=row, uops=lower(spec, ver=ver), rd1_en=True
        )
        shas[ver] = tmp.sha(ver)
    op = dve_ops.DveOp(name, spec, subdim=False, uops_sha=shas)
    dve_ops.OPS.append(op)
    dve_ops.CUSTOM_DVE_SPECS[name] = spec
    return op


def _build_bass():
    import concourse.tile as tile
    from concourse import bacc, mybir

    fp32 = mybir.dt.float32
    fp16 = mybir.dt.float16
    Alu = mybir.AluOpType
    X = mybir.AxisListType.X

    min2r = _register_min2r()
    nc = bacc.Bacc(trn_type="TRN2")

    ts13d = nc.dram_tensor("ts13", [K, TS], fp16, kind="ExternalInput")
    out_rowmin = nc.dram_tensor(
        "out_rowmin", [128, RB], fp32, kind="ExternalOutput"
    )
    # out_colmin[c, 32k + 2t + e] = min over partitions of
    # acc[:, 4096k + 256t + 2c + e]  (bitcast-fp32 transpose layout)
    out_colmin = nc.dram_tensor(
        "out_colmin", [128, M // 128], fp32, kind="ExternalOutput"
    )

    with tile.TileContext(nc) as tc:
        with (
            tc.tile_pool(name="singles", bufs=1) as singles,
            tc.tile_pool(name="dpool", bufs=2) as dpool,
            tc.tile_pool(name="folds", bufs=2) as folds,
            tc.tile_pool(name="psum", bufs=2, space="PSUM") as psum_pool,
        ):
            # fused packed operand, replicated at partition bases
            # 0/32/64/96 so the four sub-matmuls of a stripe target
            # distinct PE row groups; one DMA per replica, g=0 first so
            # the first matmuls can start as early as possible
            ts13 = singles.tile([96 + K, TS], fp16, tag="ts13")
            for g in range(4):
                eng = nc.sync if g % 2 == 0 else nc.scalar
                eng.dma_start(
                    out=ts13[32 * g : 32 * g + K, :], in_=ts13d[:, :]
                )

            identf = singles.tile([128, 128], fp32, tag="identf")
            nc.gpsimd.memset(identf, 0.0)
            nc.gpsimd.affine_select(
                out=identf,
                in_=identf,
                compare_op=Alu.not_equal,
                fill=1.0,
                base=0,
                pattern=[[-1, 128]],
                channel_multiplier=1,
            )

            # acc[p, m] = min over row blocks of D[128r+p, m]
            acc = singles.tile([128, M], fp16, tag="acc")
            rowmin = singles.tile([128, RB], fp32, tag="rowmin")
            red_all = singles.tile([128, M // 128], fp32, tag="red_all")

            # ---------------- main loop ----------------
            for j in range(RB):
                d_all = dpool.tile([128, M], fp16, tag="d_all")
                for s in range(STRIPES):
                    ps = psum_pool.tile([128, 2048], fp32, tag="ps")
                    for q in range(4):
                        g = 32 * q
                        nc.tensor.matmul(
                            ps[:, q * 512 : (q + 1) * 512],
                            ts13[g : g + K, j * 128 : (j + 1) * 128],
                            ts13[
                                g : g + K,
                                HALF
                                + s * 2048
                                + q * 512 : HALF
                                + s * 2048
                                + (q + 1) * 512,
                            ],
                            start=True,
                            stop=True,
                            tile_position=(g, 0),
                        )
                    nc.scalar.copy(
                        out=d_all[:, s * 2048 : (s + 1) * 2048], in_=ps
                    )

                # column minima accumulate: one wide fp16 2x tensor_tensor.
                # On the last iteration accumulate per half instead, so each
                # half's epilogue transposes can start while the other half
                # is still accumulating.
                if j == 0:
                    nc.vector.tensor_copy(acc, d_all)
                elif j < RB - 1:
                    nc.vector.tensor_tensor(acc, acc, d_all, op=Alu.min)
                else:
                    for h in range(2):
                        cs = slice(h * (M // 2), (h + 1) * (M // 2))
                        nc.vector.tensor_tensor(
                            acc[:, cs], acc[:, cs], d_all[:, cs], op=Alu.min
                        )

                # row minima: one fused custom DVE op (min of the two tile
                # halves elementwise, with a min-reduce accumulator)
                g1 = folds.tile([128, M // 2], fp16, tag="g1")
                nc.vector._custom_dve(
                    min2r,
                    out=g1,
                    accum_out=rowmin[:, j : j + 1],
                    in0=d_all[:, : M // 2],
                    in1=d_all[:, M // 2 :],
                    s0=BIG,
                )

            nc.sync.dma_start(out=out_rowmin[:, :], in_=rowmin)

            # ---------------- epilogue ----------------
            # col side: transpose acc bitcast as fp32 (fp16 pairs ride as
            # fp32 bit patterns, halving the transpose count), then a
            # strided free-dim reduce does the cross-partition min.
            accf = acc.bitcast(fp32)  # [128, 4096]
            for h in range(2):
                psT = psum_pool.tile([128, 16, 128], fp32, tag="ps")
                for t in range(16):
                    blk = h * 16 + t
                    nc.tensor.transpose(
                        psT[:, t, :], accf[:, blk * 128 : (blk + 1) * 128],
                        identf,
                    )
                # psT fp16 view [128, 16, 256]; reorder so the 128 source
                # partitions (stride 2) are innermost, then reduce them
                psT16 = psT.bitcast(fp16).rearrange(
                    "a b (c d) -> a b d c", d=2
                )
                nc.vector.tensor_reduce(
                    red_all[:, h * 32 : (h + 1) * 32], psT16, axis=X,
                    op=Alu.min,
                )

            nc.sync.dma_start(out=out_colmin[:, :], in_=red_all)

    nc.compile()
    return nc


def _get_nc():
    if "nc" not in _CACHE:
        _CACHE["nc"] = _build_bass()
    return _CACHE["nc"]


def _pack_operands(t, s):
    """Host-side O(N) packing: hi/lo fp16 splits + norms + ones rows.

    t: [HALF, 3] template slice, s: [M, 3] source (both fp32).
    Returns ts13 [13, HALF + M] fp16: template columns then source
    columns, with row pairing:
        t cols     s cols     product
      0-2  A1      B1         hi(-2t) . hi(s)
      3-5  A1      B2         hi(-2t) . lo(s)
      6-8  A2      B1         lo(-2t) . hi(s)
      9-10 ones    E1,E2      |s|^2 hi+lo
      11-12 nth,ntl ones      |t|^2 hi+lo
    """
    u = (-2.0 * t).T.astype(np.float32)  # [3, HALF]
    A1 = u.astype(np.float16)
    A2 = (u - A1.astype(np.float32)).astype(np.float16)
    nt = np.sum(t * t, axis=1, dtype=np.float32)  # [HALF]
    nth = nt.astype(np.float16)
    ntl = (nt - nth.astype(np.float32)).astype(np.float16)

    sv = s.T.astype(np.float32)  # [3, M]
    B1 = sv.astype(np.float16)
    B2 = (sv - B1.astype(np.float32)).astype(np.float16)
    ns = np.sum(s * s, axis=1, dtype=np.float32)  # [M]
    E1 = ns.astype(np.float16)
    E2 = (ns - E1.astype(np.float32)).astype(np.float16)

    ones_t = np.ones((2, t.shape[0]), dtype=np.float16)
    ones_s = np.ones((2, s.shape[0]), dtype=np.float16)
    t13 = np.concatenate(
        [A1, A1, A2, ones_t, nth[None, :], ntl[None, :]], axis=0
    )
    s13 = np.concatenate([B1, B2, B1, E1[None, :], E2[None, :], ones_s], axis=0)
    return np.ascontiguousarray(np.concatenate([t13, s13], axis=1))


def _make_in_maps(template, source):
    template = np.asarray(template, dtype=np.float32)
    source = np.asarray(source, dtype=np.float32)
    in_maps = []
    for c in range(N_CORES):
        b, h = divmod(c, 2)
        tmpl_half = template[b, h * HALF : (h + 1) * HALF, :]  # [HALF, 3]
        in_maps.append({"ts13": _pack_operands(tmpl_half, source[b])})
    return in_maps


def _colmin_flat(out_colmin):
    """Undo the bitcast-transpose layout: out_colmin[c, 32k + 2t + e] is
    the min of column 4096k + 256t + 2c + e. Returns [M] flat colmins."""
    v = out_colmin.reshape(128, 2, 16, 2)  # [c, k, t, e]
    return np.ascontiguousarray(
        v.transpose(1, 2, 0, 3).reshape(M)
    )  # index = 4096k + 256t + 2c + e


def _combine(results):
    # results: 8 dicts with out_rowmin [128, RB], out_colmin [128, M//128]
    row_total = 0.0
    col_total = 0.0
    for b in range(B):
        r0 = results[2 * b]
        r1 = results[2 * b + 1]
        for r in (r0, r1):
            rm = np.maximum(r["out_rowmin"].astype(np.float64), 0.0)
            row_total += float(np.sum(np.sqrt(rm)))
        c0 = _colmin_flat(r0["out_colmin"])
        c1 = _colmin_flat(r1["out_colmin"])
        cm = np.maximum(np.minimum(c0, c1).astype(np.float64), 0.0)
        col_total += float(np.sum(np.sqrt(cm)))
    loss = (row_total + col_total) / (2.0 * B * float(N))
    return np.float32(loss)


def _run_on_cores(in_maps, trace=False, **kwargs):
    from concourse.bass_utils import run_bass_kernel_spmd

    nc = _get_nc()
    return run_bass_kernel_spmd(
        nc, in_maps, core_ids=list(range(N_CORES)), trace=trace, **kwargs
    )


def kernel(template, source):
    in_maps = _make_in_maps(template, source)
    res = _run_on_cores(in_maps, trace=False)
    return _combine(res.results)


# revision 10
# speedup vs baseline: 1.3719x; 1.0249x over previous
"""Chamfer distance loss kernel for Trainium2 (8 NeuronCores).

Problem: template/source [4, 8192, 3] fp32 -> scalar chamfer loss.

Sharding: 8 cores = 4 batches x 2 template-halves. Each core computes the
[4096, 8192] squared-distance matrix D between its template half and the
full source of its batch:
    d[n,m] = |t_n|^2 + |s_m|^2 - 2 t_n . s_m

All K=13 terms ride a single fp16 matmul so PSUM holds the COMPLETE D:
the three first-order cross blocks of the hi/lo fp16 split of u=-2t and
s (~22 mantissa bits combined), |s|^2 hi/lo against template-side ones
rows, and |t|^2 hi/lo against source-side ones rows. The packed operand
image (norms + hi/lo splits are O(N) work) is built on the HOST in
numpy; the device prologue is 4 replica DMA loads.

The packed image is replicated at partition bases 0/32/64/96 and the
four 512-column sub-matmuls of each stripe use different bases, so the
matmuls run concurrently in distinct PE row groups.

Main loop per template row block j (32 iterations, [128, 8192] D row):
  - ScalarE: d16 = fp16(psum), four [128, 2048] casts (pure drain).
  - VectorE: one wide fp16 2x tensor_tensor min accumulates column
    minima; one custom fused DVE op (MIN2R: out = min(lo, hi) halves,
    accum_out = free-dim min) produces the complete row minimum.
  - TensorE epilogue: transpose the column accumulator as BITCAST fp32
    (halves the transpose count; fp16 pairs ride as fp32 bit patterns)
    into PSUM; a strided free-dim reduce does the cross-partition min.

Host combine: sqrt/clamp/sums of the tiny per-core min arrays (O(N)),
plus the elementwise min over the two half-core column arrays.
"""

import numpy as np

B = 4
N = 8192  # template points per batch
M = 8192  # source points per batch
HALF = N // 2  # template rows per core
RB = HALF // 128  # 32 row blocks per core
STRIPES = M // 2048  # 4 col stripes of 2048
K = 13  # packed contraction dim
TS = HALF + M  # fused operand image columns (template then source)
N_CORES = 8
BIG = 60000.0  # > any real distance, < fp16 max

_CACHE = {}


def _register_min2r():
    """Register a fused custom DVE op: out = min(in0, in1) elementwise,
    accum_out = min(s0, min over free dim of out). One instruction reduces
    two [128, 4096] fp16 tiles to a per-partition row minimum (~4.4us),
    replacing a five-op fold tree (~5.5us)."""
    import concourse.dve_ops as dve_ops
    from concourse.dve_spec import Spec, Src0, Src1, minn, C0, lower, AluOp
    from concourse.dve_uop import DveOpSpec

    name = "MIN2R_CHAMFER"
    for o in dve_ops.OPS:
        if o.name == name:
            return o
    row = max(dve_ops._SUB_OPCODE_FOR_NAME.values()) + 1
    assert row < 0x20
    spec = Spec(body=minn(Src0, Src1), accum=AluOp.MIN, accum_init=C0)
    dve_ops._SUB_OPCODE_FOR_NAME[name] = row
    shas = {}
    for ver in ("v3", "v4"):
        tmp = DveOpSpec(
            name=name, opcode=row, uops=lower(spec, ver=ver), rd1_en=True
        )
        shas[ver] = tmp.sha(ver)
    op = dve_ops.DveOp(name, spec, subdim=False, uops_sha=shas)
    dve_ops.OPS.append(op)
    dve_ops.CUSTOM_DVE_SPECS[name] = spec
    return op


def _build_bass():
    import concourse.tile as tile
    from concourse import bacc, mybir

    fp32 = mybir.dt.float32
    fp16 = mybir.dt.float16
    Alu = mybir.AluOpType
    X = mybir.AxisListType.X

    min2r = _register_min2r()
    nc = bacc.Bacc(trn_type="TRN2")

    ts13d = nc.dram_tensor("ts13", [K, TS], fp16, kind="ExternalInput")
    out_rowmin = nc.dram_tensor(
        "out_rowmin", [128, RB], fp32, kind="ExternalOutput"
    )
    # out_colmin[c, 32k + 2t + e] = min over partitions of
    # acc[:, 4096k + 256t + 2c + e]  (bitcast-fp32 transpose layout)
    out_colmin = nc.dram_tensor(
        "out_colmin", [128, M // 128], fp32, kind="ExternalOutput"
    )

    with tile.TileContext(nc) as tc:
        with (
            tc.tile_pool(name="singles", bufs=1) as singles,
            tc.tile_pool(name="dpool", bufs=2) as dpool,
            tc.tile_pool(name="folds", bufs=2) as folds,
            tc.tile_pool(name="psum", bufs=2, space="PSUM") as psum_pool,
        ):
            # fused packed operand, replicated at partition bases
            # 0/32/64/96 so the four sub-matmuls of a stripe target
            # distinct PE row groups; one DMA per replica, g=0 first so
            # the first matmuls can start as early as possible
            ts13 = singles.tile([96 + K, TS], fp16, tag="ts13")
            for g in range(4):
                eng = nc.sync if g % 2 == 0 else nc.scalar
                eng.dma_start(
                    out=ts13[32 * g : 32 * g + K, :], in_=ts13d[:, :]
                )

            identf = singles.tile([128, 128], fp32, tag="identf")
            nc.gpsimd.memset(identf, 0.0)
            nc.gpsimd.affine_select(
                out=identf,
                in_=identf,
                compare_op=Alu.not_equal,
                fill=1.0,
                base=0,
                pattern=[[-1, 128]],
                channel_multiplier=1,
            )

            # acc[p, m] = min over row blocks of D[128r+p, m]
            acc = singles.tile([128, M], fp16, tag="acc")
            rowmin = singles.tile([128, RB], fp32, tag="rowmin")
            red_all = singles.tile([128, M // 128], fp32, tag="red_all")

            # ---------------- main loop ----------------
            for j in range(RB):
                d_all = dpool.tile([128, M], fp16, tag="d_all")
                for s in range(STRIPES):
                    ps = psum_pool.tile([128, 2048], fp32, tag="ps")
                    for q in range(4):
                        # j == 0 runs entirely in row group 0, which only
                        # needs the first replica DMA - the pipeline starts
                        # ~13us before the other replicas finish loading
                        g = 0 if j == 0 else 32 * q
                        nc.tensor.matmul(
                            ps[:, q * 512 : (q + 1) * 512],
                            ts13[g : g + K, j * 128 : (j + 1) * 128],
                            ts13[
                                g : g + K,
                                HALF
                                + s * 2048
                                + q * 512 : HALF
                                + s * 2048
                                + (q + 1) * 512,
                            ],
                            start=True,
                            stop=True,
                            tile_position=(g, 0),
                        )
                    nc.scalar.copy(
                        out=d_all[:, s * 2048 : (s + 1) * 2048], in_=ps
                    )

                # column minima accumulate: one wide fp16 2x tensor_tensor
                if j == 0:
                    nc.vector.tensor_copy(acc, d_all)
                else:
                    nc.vector.tensor_tensor(acc, acc, d_all, op=Alu.min)

                # row minima: one fused custom DVE op (min of the two tile
                # halves elementwise, with a min-reduce accumulator). At
                # j = RB-1 this runs after the col accumulate and overlaps
                # the epilogue transposes.
                g1 = folds.tile([128, M // 2], fp16, tag="g1")
                nc.vector._custom_dve(
                    min2r,
                    out=g1,
                    accum_out=rowmin[:, j : j + 1],
                    in0=d_all[:, : M // 2],
                    in1=d_all[:, M // 2 :],
                    s0=BIG,
                )

            nc.sync.dma_start(out=out_rowmin[:, :], in_=rowmin)

            # ---------------- epilogue ----------------
            # col side: transpose acc bitcast as fp32 (fp16 pairs ride as
            # fp32 bit patterns, halving the transpose count), then a
            # strided free-dim reduce does the cross-partition min; four
            # chunks so each reduce overlaps the next chunk's transposes.
            accf = acc.bitcast(fp32)  # [128, 4096]
            for h in range(4):
                psT = psum_pool.tile([128, 8, 128], fp32, tag="ps")
                for t in range(8):
                    blk = h * 8 + t
                    nc.tensor.transpose(
                        psT[:, t, :], accf[:, blk * 128 : (blk + 1) * 128],
                        identf,
                    )
                # psT fp16 view [128, 8, 256]; reorder so the 128 source
                # partitions (stride 2) are innermost, then reduce them
                psT16 = psT.bitcast(fp16).rearrange(
                    "a b (c d) -> a b d c", d=2
                )
                nc.vector.tensor_reduce(
                    red_all[:, h * 16 : (h + 1) * 16], psT16, axis=X,
                    op=Alu.min,
                )

            nc.sync.dma_start(out=out_colmin[:, :], in_=red_all)

    nc.compile()
    return nc


def _get_nc():
    if "nc" not in _CACHE:
        _CACHE["nc"] = _build_bass()
    return _CACHE["nc"]


def _pack_operands(t, s):
    """Host-side O(N) packing: hi/lo fp16 splits + norms + ones rows.

    t: [HALF, 3] template slice, s: [M, 3] source (both fp32).
    Returns ts13 [13, HALF + M] fp16: template columns then source
    columns, with row pairing:
        t cols     s cols     product
      0-2  A1      B1         hi(-2t) . hi(s)
      3-5  A1      B2         hi(-2t) . lo(s)
      6-8  A2      B1         lo(-2t) . hi(s)
      9-10 ones    E1,E2      |s|^2 hi+lo
      11-12 nth,ntl ones      |t|^2 hi+lo
    """
    u = (-2.0 * t).T.astype(np.float32)  # [3, HALF]
    A1 = u.astype(np.float16)
    A2 = (u - A1.astype(np.float32)).astype(np.float16)
    nt = np.sum(t * t, axis=1, dtype=np.float32)  # [HALF]
    nth = nt.astype(np.float16)
    ntl = (nt - nth.astype(np.float32)).astype(np.float16)

    sv = s.T.astype(np.float32)  # [3, M]
    B1 = sv.astype(np.float16)
    B2 = (sv - B1.astype(np.float32)).astype(np.float16)
    ns = np.sum(s * s, axis=1, dtype=np.float32)  # [M]
    E1 = ns.astype(np.float16)
    E2 = (ns - E1.astype(np.float32)).astype(np.float16)

    ones_t = np.ones((2, t.shape[0]), dtype=np.float16)
    ones_s = np.ones((2, s.shape[0]), dtype=np.float16)
    t13 = np.concatenate(
        [A1, A1, A2, ones_t, nth[None, :], ntl[None, :]], axis=0
    )
    s13 = np.concatenate([B1, B2, B1, E1[None, :], E2[None, :], ones_s], axis=0)
    return np.ascontiguousarray(np.concatenate([t13, s13], axis=1))


def _make_in_maps(template, source):
    template = np.asarray(template, dtype=np.float32)
    source = np.asarray(source, dtype=np.float32)
    in_maps = []
    for c in range(N_CORES):
        b, h = divmod(c, 2)
        tmpl_half = template[b, h * HALF : (h + 1) * HALF, :]  # [HALF, 3]
        in_maps.append({"ts13": _pack_operands(tmpl_half, source[b])})
    return in_maps


def _colmin_flat(out_colmin):
    """Undo the bitcast-transpose layout: out_colmin[c, 32k + 2t + e] is
    the min of column 4096k + 256t + 2c + e. Returns [M] flat colmins."""
    v = out_colmin.reshape(128, 2, 16, 2)  # [c, k, t, e]
    return np.ascontiguousarray(
        v.transpose(1, 2, 0, 3).reshape(M)
    )  # index = 4096k + 256t + 2c + e


def _combine(results):
    # results: 8 dicts with out_rowmin [128, RB], out_colmin [128, M//128]
    row_total = 0.0
    col_total = 0.0
    for b in range(B):
        r0 = results[2 * b]
        r1 = results[2 * b + 1]
        for r in (r0, r1):
            rm = np.maximum(r["out_rowmin"].astype(np.float64), 0.0)
            row_total += float(np.sum(np.sqrt(rm)))
        c0 = _colmin_flat(r0["out_colmin"])
        c1 = _colmin_flat(r1["out_colmin"])
        cm = np.maximum(np.minimum(c0, c1).astype(np.float64), 0.0)
        col_total += float(np.sum(np.sqrt(cm)))
    loss = (row_total + col_total) / (2.0 * B * float(N))
    return np.float32(loss)


def _run_on_cores(in_maps, trace=False, **kwargs):
    from concourse.bass_utils import run_bass_kernel_spmd

    nc = _get_nc()
    return run_bass_kernel_spmd(
        nc, in_maps, core_ids=list(range(N_CORES)), trace=trace, **kwargs
    )


def kernel(template, source):
    in_maps = _make_in_maps(template, source)
    res = _run_on_cores(in_maps, trace=False)
    return _combine(res.results)
